# revision 1
# baseline (speedup 1.0000x reference)
import numpy as np

# DGCNN (2x DynamicEdgeConv + global max pool + MLP head) on 8 NeuronCores.
# Data-parallel over jets (512 -> 64/core); BN batch statistics made exact
# via 3 tiny AllReduces of per-partition moment sums.

N_CORES = 8
B, N, F = 512, 128, 16
J = B // N_CORES          # 64 jets per core
K = 20                     # neighbors used
NG1 = 4                    # conv1 jet-stack (4 x 32ch)
NG2 = 2                    # conv2 jet-stack (2 x 64ch)
G1 = J // NG1              # 16 groups conv1
G2 = J // NG2              # 32 groups conv2
E = 2560                   # K*N edges per jet
EPS = 1e-5
NEG = -1.0e30


def _blockdiag(w, n):
    k, m = w.shape
    out = np.zeros((k * n, m * n), np.float32)
    for i in range(n):
        out[i * k:(i + 1) * k, i * m:(i + 1) * m] = w
    return out


def _build_nc():
    import concourse.bass as bass
    import concourse.mybir as mybir
    import concourse.tile as tile
    from concourse import bacc

    fp32 = mybir.dt.float32
    AF = mybir.ActivationFunctionType
    OP = mybir.AluOpType

    nc = bacc.Bacc(None)

    def din(name, shape):
        return nc.dram_tensor(name, shape, fp32, kind="ExternalInput")

    ptsA = din("ptsA", [4, J, N])
    ptsB = din("ptsB", [4, J, N])
    featC = din("featC", [16, J, N])
    w1a = din("w1a", [16, 32])
    w1b = din("w1b", [16, 32])
    w2bd = din("w2bd", [128, 128])
    w3bd = din("w3bd", [128, 128])
    w2pa = din("w2pa", [32, 64])
    w2pb = din("w2pb", [32, 64])
    w2pbd = din("w2pbd", [128, 128])
    mh1 = din("mh1", [64, 128])
    mh2 = din("mh2", [128, 128])
    mh3 = din("mh3", [128, 8])
    g1r = din("g1r", [128, 1])
    be1r = din("be1r", [128, 1])
    g2r = din("g2r", [128, 1])
    be2r = din("be2r", [128, 1])
    b3r = din("b3r", [128, 1])
    g3r = din("g3r", [128, 1])
    be3r = din("be3r", [128, 1])
    b2pr = din("b2pr", [128, 1])
    mb1 = din("mb1", [128, 1])
    mb2 = din("mb2", [128, 1])
    mb3 = din("mb3", [1, 1])
    fold4 = din("fold4", [128, 128])
    onesrow = din("onesrow", [1, 128])
    fold2 = din("fold2", [128, 128])
    i432 = din("i432", [128, 32])

    out_t = nc.dram_tensor("out", [1, J], fp32, kind="ExternalOutput")

    cc_in = [nc.dram_tensor(f"cc_in{i}", [128, 4], fp32) for i in range(3)]
    cc_out = [nc.dram_tensor(f"cc_out{i}", [128, 4], fp32) for i in range(3)]

    i16 = mybir.dt.int16
    u32 = mybir.dt.uint32

    with tile.TileContext(nc) as tc:
        with (
            tc.tile_pool(name="persist", bufs=1) as P,
            tc.tile_pool(name="work", bufs=2) as W,
            tc.tile_pool(name="small", bufs=4) as S,
            tc.tile_pool(name="inp", bufs=2) as SI,
            tc.tile_pool(name="gatp", bufs=3) as WG,
            tc.tile_pool(name="stats", bufs=1) as ST,
            tc.tile_pool(name="psum", bufs=3, space="PSUM") as PS,
            tc.tile_pool(name="psum2", bufs=2, space="PSUM") as PS2,
            tc.tile_pool(name="psum3", bufs=3, space="PSUM") as PS3,
        ):
            # ---- load constants/weights ----
            def load(t, shape=None):
                sb = P.tile(shape or list(t.shape), fp32, tag=f"ld_{t.name}")
                nc.sync.dma_start(out=sb, in_=t[:, :] if len(t.shape) == 2 else t[:, :, :])
                return sb

            w1a_s = load(w1a); w1b_s = load(w1b)
            w2bd_s = load(w2bd); w3bd_s = load(w3bd)
            w2pa_s = load(w2pa); w2pb_s = load(w2pb); w2pbd_s = load(w2pbd)
            mh1_s = load(mh1); mh2_s = load(mh2); mh3_s = load(mh3)
            fold4_s = load(fold4); fold2_s = load(fold2); i432_s = load(i432)
            g1r_s = load(g1r); be1r_s = load(be1r); g2r_s = load(g2r); be2r_s = load(be2r)
            b3r_s = load(b3r); g3r_s = load(g3r); be3r_s = load(be3r); b2pr_s = load(b2pr)
            mb1_s = load(mb1); mb2_s = load(mb2); mb3_s = load(mb3)

            epsap = P.tile([128, 1], fp32, tag="epsap")
            nc.vector.memset(epsap, EPS)

            # ---- persistent intermediates ----
            IdxBig = P.tile([128, J, 24], u32, tag="IdxBig")
            Idx16 = P.tile([128, J, 20], i16, tag="Idx16")
            IdxW1 = P.tile([128, G1, 160], i16, tag="IdxW1")
            IdxW2 = P.tile([128, G2, 160], i16, tag="IdxW2")
            Bm1_all = P.tile([128, G1, N], fp32, tag="Bm1_all")
            SH1_all = P.tile([128, G1, N], fp32, tag="SH1_all")
            B2_all = P.tile([128, G2, N], fp32, tag="B2_all")
            SH2_all = P.tile([128, G2, N], fp32, tag="SH2_all")
            x1_all = P.tile([128, G1, N], fp32, tag="x1_all")
            pooled = P.tile([128, G2], fp32, tag="pooled")

            def topk_jet(score_ps, g):
                # score = -D in SBUF, 3 rounds of max8/max_index/match_replace
                sc = W.tile([128, N], fp32, tag="score")
                nc.scalar.activation(out=sc, in_=score_ps, func=AF.Copy, scale=-1.0)
                for r in range(3):
                    m8 = S.tile([128, 8], fp32, tag="m8")
                    nc.vector.max(out=m8, in_=sc)
                    nc.vector.max_index(
                        out=IdxBig[:, g, r * 8:(r + 1) * 8], in_max=m8, in_values=sc)
                    if r < 2:
                        nc.vector.match_replace(
                            out=sc, in_to_replace=m8, in_values=sc, imm_value=NEG)

            # ---- phase A: conv1 knn (inputs batched 8 jets/DMA) ----
            for blk in range(J // 8):
                pA = SI.tile([4, 8, N], fp32, tag="pA")
                pB = SI.tile([4, 8, N], fp32, tag="pB")
                nc.sync.dma_start(out=pA, in_=ptsA[:, blk * 8:(blk + 1) * 8, :])
                nc.sync.dma_start(out=pB, in_=ptsB[:, blk * 8:(blk + 1) * 8, :])
                for j in range(8):
                    g = blk * 8 + j
                    psD = PS.tile([128, N], fp32, tag="ps")
                    nc.tensor.matmul(psD, pA[:, j, :], pB[:, j, :],
                                     start=True, stop=True)
                    topk_jet(psD, g)

            # ---- phase B: conv1 L1 ----
            for blk in range(J // 8):
                fC = SI.tile([16, 8, N], fp32, tag="fC")
                nc.sync.dma_start(out=fC, in_=featC[:, blk * 8:(blk + 1) * 8, :])
                for j in range(8):
                    g = blk * 8 + j
                    k = g % NG1
                    grp = g // NG1
                    psA = PS.tile([128, N], fp32, tag="ps")
                    psB = PS.tile([128, N], fp32, tag="ps")
                    nc.tensor.matmul(psA[k * 32:(k + 1) * 32, :], w1a_s, fC[:, j, :],
                                     start=True, stop=True, tile_position=(0, k * 32))
                    nc.tensor.matmul(psB[k * 32:(k + 1) * 32, :], w1b_s, fC[:, j, :],
                                     start=True, stop=True, tile_position=(0, k * 32))
                    nc.scalar.copy(out=Bm1_all[k * 32:(k + 1) * 32, grp, :],
                                   in_=psB[k * 32:(k + 1) * 32, :])
                    nc.vector.tensor_sub(out=SH1_all[k * 32:(k + 1) * 32, grp, :],
                                         in0=psA[k * 32:(k + 1) * 32, :],
                                         in1=Bm1_all[k * 32:(k + 1) * 32, grp, :])

            def wrap_idx(IdxW, ng):
                # cast uint32 -> int16 (values < 128)
                nc.vector.tensor_copy(out=Idx16[:, :, :], in_=IdxBig[:, :, 1:21])
                # per-jet wrap DMAs into 16-row groups (replicated per 32/64-ch band)
                nreps = 32 // 16 if ng == NG1 else 64 // 16
                for g in range(J):
                    grp = g // ng
                    k = g % ng
                    band = k * (32 if ng == NG1 else 64)
                    for rep in range(nreps):
                        base = band + rep * 16
                        nc.sync.dma_start(
                            out=IdxW[base:base + 16, grp, :],
                            in_=Idx16[:, g, :])

            wrap_idx(IdxW1, NG1)

            def edge_pass(src, SH, IdxW, ngrp, mode, s1=None, t1=None,
                          s2=None, t2=None, stats_t=None, wfold=None,
                          xout=None, b3ap=None, s3=None, t3=None):
                for grp in range(ngrp):
                    gat = WG.tile([128, E], fp32, tag="gat")
                    nc.gpsimd.ap_gather(
                        out_ap=gat[:, :], in_ap=src[:, grp, :],
                        idxs_ap=IdxW[:, grp, :],
                        channels=128, num_elems=N, d=1, num_idxs=E)
                    shv = SH[:, grp, :]
                    import concourse.bass as _b
                    sh_b = _b.AP(tensor=shv.tensor, offset=shv.offset,
                                 ap=[shv.ap[0], [1, 8], [0, 20], [8, 16]])
                    g4 = gat.rearrange("p (a b q) -> p a b q", b=20, q=16)
                    nc.vector.tensor_add(out=g4, in0=g4, in1=sh_b)
                    if mode == "stats1":
                        for c in range(5):
                            nc.vector.bn_stats(
                                out=stats_t[:, grp * 5 + c, :],
                                in_=gat[:, c * 512:(c + 1) * 512])
                        continue
                    r1 = W.tile([128, E], fp32, tag="r1")
                    nc.scalar.activation(out=r1, in_=gat, func=AF.Relu,
                                         bias=t1, scale=s1)
                    if mode == "final2":
                        # conv2: fold mean through last linear
                        psx = PS2.tile([128, N], fp32, tag="psx")
                        r4 = r1.rearrange("p (a b q) -> p a b q", b=20, q=16)
                        for bb in range(20):
                            nc.tensor.matmul(psx, wfold, r4[:, :, bb, :],
                                             start=(bb == 0), stop=(bb == 19))
                        pm = S.tile([128, 1], fp32, tag="pm")
                        nc.vector.tensor_reduce(out=pm, in_=psx,
                                                axis=mybir.AxisListType.X, op=OP.max)
                        nc.vector.tensor_scalar(out=pooled[:, grp:grp + 1], in0=pm,
                                                scalar1=1.0 / K, scalar2=b3ap,
                                                op0=OP.mult, op1=OP.add)
                        continue
                    # conv1: L2 matmul, chunk-sequential to bound PSUM use
                    r2 = None
                    if mode == "final1":
                        r2 = W.tile([128, E], fp32, tag="r2")
                    for c in range(5):
                        p = PS3.tile([128, 512], fp32, tag="psh")
                        nc.tensor.matmul(p, w2bd_s, r1[:, c * 512:(c + 1) * 512],
                                         start=True, stop=True)
                        if mode == "stats2":
                            nc.vector.bn_stats(out=stats_t[:, grp * 5 + c, :],
                                               in_=p)
                        else:
                            nc.scalar.activation(out=r2[:, c * 512:(c + 1) * 512],
                                                 in_=p, func=AF.Relu,
                                                 bias=t2, scale=s2)
                    if mode == "stats2":
                        continue
                    psx = PS2.tile([128, N], fp32, tag="psx")
                    r4 = r2.rearrange("p (a b q) -> p a b q", b=20, q=16)
                    for bb in range(20):
                        nc.tensor.matmul(psx, wfold, r4[:, :, bb, :],
                                         start=(bb == 0), stop=(bb == 19))
                    nc.vector.tensor_scalar(out=xout[:, grp, :], in0=psx,
                                            scalar1=1.0 / K, scalar2=b3ap,
                                            op0=OP.mult, op1=OP.add)

            def bn_param(stats_t, nchunk, foldm, gam, bet, cci, cco, nunits):
                # aggregate local stats -> (m, v, m2) -> AllReduce -> fold -> s,t
                mv = S.tile([128, 2], fp32, tag="mv")
                if nchunk <= 80:
                    nc.vector.bn_aggr(out=mv, in_=stats_t[:, 0:nchunk, :])
                else:
                    h = nchunk // 2
                    mv1 = S.tile([128, 2], fp32, tag="mv1")
                    mv2 = S.tile([128, 2], fp32, tag="mv2")
                    nc.vector.bn_aggr(out=mv1, in_=stats_t[:, 0:h, :])
                    nc.vector.bn_aggr(out=mv2, in_=stats_t[:, h:nchunk, :])
                    # equal halves: m=(m1+m2)/2 ; v=(v1+v2)/2+((m1-m2)/2)^2
                    d = S.tile([128, 1], fp32, tag="mvd")
                    nc.vector.tensor_sub(out=d, in0=mv1[:, 0:1], in1=mv2[:, 0:1])
                    nc.vector.tensor_scalar_mul(out=d, in0=d, scalar1=0.5)
                    nc.vector.tensor_mul(out=d, in0=d, in1=d)
                    nc.vector.tensor_add(out=mv[:, 0:1], in0=mv1[:, 0:1], in1=mv2[:, 0:1])
                    nc.vector.tensor_scalar_mul(out=mv[:, 0:1], in0=mv[:, 0:1], scalar1=0.5)
                    nc.vector.tensor_add(out=mv[:, 1:2], in0=mv1[:, 1:2], in1=mv2[:, 1:2])
                    nc.vector.tensor_scalar(out=mv[:, 1:2], in0=mv[:, 1:2],
                                            scalar1=0.5, scalar2=None, op0=OP.mult)
                    nc.vector.tensor_add(out=mv[:, 1:2], in0=mv[:, 1:2], in1=d)
                pay = S.tile([128, 4], fp32, tag="pay")
                nc.vector.tensor_copy(out=pay[:, 0:2], in_=mv)
                nc.vector.tensor_mul(out=pay[:, 2:3], in0=mv[:, 0:1], in1=mv[:, 0:1])
                nc.vector.memset(pay[:, 3:4], 0.0)
                nc.gpsimd.dma_start(out=cci[:, :], in_=pay)
                nc.gpsimd.collective_compute(
                    "AllReduce", OP.add,
                    replica_groups=[list(range(N_CORES))],
                    ins=[cci[:, :]], outs=[cco[:, :]])
                arr = S.tile([128, 4], fp32, tag="arr")
                nc.gpsimd.dma_start(out=arr, in_=cco[:, :])
                psf = PS.tile([128, 4], fp32, tag="ps")
                nc.tensor.matmul(psf, foldm, arr, start=True, stop=True)
                mg = S.tile([128, 1], fp32, tag="mg")
                vg = S.tile([128, 1], fp32, tag="vg")
                nc.vector.tensor_scalar_mul(out=mg, in0=psf[:, 0:1], scalar1=1.0 / nunits)
                m2g = S.tile([128, 1], fp32, tag="m2g")
                nc.vector.tensor_scalar_mul(out=m2g, in0=psf[:, 2:3], scalar1=1.0 / nunits)
                nc.vector.tensor_scalar_mul(out=vg, in0=psf[:, 1:2], scalar1=1.0 / nunits)
                nc.vector.tensor_add(out=vg, in0=vg, in1=m2g)
                mm = S.tile([128, 1], fp32, tag="mm")
                nc.vector.tensor_mul(out=mm, in0=mg, in1=mg)
                nc.vector.tensor_sub(out=vg, in0=vg, in1=mm)
                sd = S.tile([128, 1], fp32, tag="sd")
                nc.scalar.activation(out=sd, in_=vg, func=AF.Sqrt, bias=epsap, scale=1.0)
                ri = S.tile([128, 1], fp32, tag="ri")
                nc.vector.reciprocal(out=ri, in_=sd)
                s = P.tile([128, 1], fp32, tag=f"bn_s_{cci.name}")
                t = P.tile([128, 1], fp32, tag=f"bn_t_{cci.name}")
                nc.vector.tensor_mul(out=s, in0=gam, in1=ri)
                nc.vector.tensor_mul(out=t, in0=mg, in1=s)
                nc.vector.tensor_sub(out=t, in0=bet, in1=t)
                return s, t

            # ---- conv1 stats + passes ----
            stats1 = ST.tile([128, G1 * 5, 6], fp32, tag="stats1")
            edge_pass(Bm1_all, SH1_all, IdxW1, G1, "stats1", stats_t=stats1)
            s1, t1 = bn_param(stats1, G1 * 5, fold4_s, g1r_s, be1r_s,
                              cc_in[0], cc_out[0], 4 * N_CORES)
            stats2 = ST.tile([128, G1 * 5, 6], fp32, tag="stats2")
            edge_pass(Bm1_all, SH1_all, IdxW1, G1, "stats2", s1=s1, t1=t1,
                      stats_t=stats2)
            s2, t2 = bn_param(stats2, G1 * 5, fold4_s, g2r_s, be2r_s,
                              cc_in[1], cc_out[1], 4 * N_CORES)
            edge_pass(Bm1_all, SH1_all, IdxW1, G1, "final1", s1=s1, t1=t1,
                      s2=s2, t2=t2, wfold=w3bd_s, xout=x1_all, b3ap=b3r_s)

            # ---- conv2 prep ----
            ones32 = P.tile([32, 1], fp32, tag="ones32")
            nc.vector.memset(ones32, 1.0)
            for g in range(J):
                k1 = g % NG1
                grp1 = g // NG1
                k2 = g % NG2
                grp2 = g // NG2
                augA = W.tile([34, N], fp32, tag="augA")
                augB = W.tile([34, N], fp32, tag="augB")
                nc.sync.dma_start(out=augA[0:32, :],
                                  in_=x1_all[k1 * 32:(k1 + 1) * 32, grp1, :])
                sq = W.tile([32, N], fp32, tag="sq")
                nc.scalar.square(out=sq, in_=augA[0:32, :])
                pssq = PS.tile([1, N], fp32, tag="ps")
                nc.tensor.matmul(pssq, ones32, sq, start=True, stop=True)
                sqr = S.tile([1, N], fp32, tag="sqr")
                nc.scalar.copy(out=sqr, in_=pssq)
                nc.sync.dma_start(out=augA[32:33, :], in_=sqr)
                nc.sync.dma_start(out=augA[33:34, :], in_=onesrow[:, :])
                nc.scalar.activation(out=augB[0:32, :], in_=augA[0:32, :],
                                     func=AF.Copy, scale=-2.0)
                nc.sync.dma_start(out=augB[32:33, :], in_=onesrow[:, :])
                nc.sync.dma_start(out=augB[33:34, :], in_=sqr)
                psD = PS.tile([128, N], fp32, tag="ps")
                nc.tensor.matmul(psD, augA, augB, start=True, stop=True)
                topk_jet(psD, g)
                # L1' A2/B2 into 64-row band via col tiling
                psA2 = PS.tile([128, N], fp32, tag="ps")
                psB2 = PS.tile([128, N], fp32, tag="ps")
                base = k2 * 64
                nc.tensor.matmul(psA2[base:base + 64, :], w2pa_s, augA[0:32, :],
                                 start=True, stop=True, tile_position=(0, base))
                nc.tensor.matmul(psB2[base:base + 64, :], w2pb_s, augA[0:32, :],
                                 start=True, stop=True, tile_position=(0, base))
                nc.scalar.copy(out=B2_all[base:base + 64, grp2, :],
                               in_=psB2[base:base + 64, :])
                nc.vector.tensor_sub(out=SH2_all[base:base + 64, grp2, :],
                                     in0=psA2[base:base + 64, :],
                                     in1=B2_all[base:base + 64, grp2, :])

            wrap_idx(IdxW2, NG2)

            # ---- conv2 stats + final ----
            stats3 = ST.tile([128, G2 * 5, 6], fp32, tag="stats3")
            edge_pass(B2_all, SH2_all, IdxW2, G2, "stats1", stats_t=stats3)
            s3, t3 = bn_param(stats3, G2 * 5, fold2_s, g3r_s, be3r_s,
                              cc_in[2], cc_out[2], 2 * N_CORES)
            edge_pass(B2_all, SH2_all, IdxW2, G2, "final2", s1=s3, t1=t3,
                      wfold=w2pbd_s, b3ap=b2pr_s)

            # ---- head ----
            Gh = P.tile([64, J], fp32, tag="Gh")
            gh_v = Gh.rearrange("p (g s) -> p g s", s=2)
            nc.sync.dma_start(out=gh_v[:, :, 0], in_=pooled[0:64, :])
            nc.sync.dma_start(out=gh_v[:, :, 1], in_=pooled[64:128, :])
            ps1 = PS.tile([128, J], fp32, tag="ps")
            nc.tensor.matmul(ps1, mh1_s, Gh, start=True, stop=True)
            hh1 = W.tile([128, J], fp32, tag="hh1")
            nc.scalar.activation(out=hh1, in_=ps1, func=AF.Relu, bias=mb1_s, scale=1.0)
            ps2 = PS.tile([128, J], fp32, tag="ps")
            nc.tensor.matmul(ps2, mh2_s, hh1, start=True, stop=True)
            hh2 = W.tile([128, J], fp32, tag="hh2")
            nc.scalar.activation(out=hh2, in_=ps2, func=AF.Relu, bias=mb2_s, scale=1.0)
            ps3 = PS.tile([8, J], fp32, tag="ps")
            nc.tensor.matmul(ps3, mh3_s, hh2, start=True, stop=True)
            ov = W.tile([1, J], fp32, tag="ov")
            nc.vector.tensor_scalar(out=ov, in0=ps3[0:1, :], scalar1=mb3_s[0:1, 0:1],
                                    scalar2=None, op0=OP.add)
            nc.sync.dma_start(out=out_t[:, :], in_=ov)

    nc.finalize()
    return nc


_NC_CACHE = None


def kernel(**inputs) -> np.ndarray:
    global _NC_CACHE
    from concourse.bass_utils import run_bass_kernel_spmd

    if _NC_CACHE is None:
        _NC_CACHE = _build_nc()
    nc = _NC_CACHE

    pts = inputs["points"].astype(np.float32)
    feat = inputs["features"].astype(np.float32)

    w1 = inputs["c1_w1"]
    shared = {
        "w1a": np.ascontiguousarray(w1[:16]), "w1b": np.ascontiguousarray(w1[16:]),
        "w2bd": _blockdiag(inputs["c1_w2"], 4),
        "w3bd": _blockdiag(inputs["c1_w3"], 4),
        "w2pa": np.ascontiguousarray(inputs["c2_w1"][:32]),
        "w2pb": np.ascontiguousarray(inputs["c2_w1"][32:]),
        "w2pbd": _blockdiag(inputs["c2_w2"], 2),
        "mh1": inputs["m_w1"].astype(np.float32),
        "mh2": inputs["m_w2"].astype(np.float32),
        "mh3": np.pad(inputs["m_w3"].astype(np.float32), ((0, 0), (0, 7))),
        "g1r": np.tile(inputs["c1_g1"], 4)[:, None],
        "be1r": np.tile(inputs["c1_be1"], 4)[:, None],
        "g2r": np.tile(inputs["c1_g2"], 4)[:, None],
        "be2r": np.tile(inputs["c1_be2"], 4)[:, None],
        "b3r": np.tile(inputs["c1_b3"], 4)[:, None],
        "g3r": np.tile(inputs["c2_g1"], 2)[:, None],
        "be3r": np.tile(inputs["c2_be1"], 2)[:, None],
        "b2pr": np.tile(inputs["c2_b2"], 2)[:, None],
        "mb1": inputs["m_b1"].astype(np.float32)[:, None],
        "mb2": inputs["m_b2"].astype(np.float32)[:, None],
        "mb3": inputs["m_b3"].astype(np.float32)[:, None],
        "i432": np.tile(np.eye(32, dtype=np.float32), (4, 1)),
    }
    pp = np.arange(128)
    f4 = (pp[:, None] % 32 == pp[None, :] % 32).astype(np.float32)
    f2 = (pp[:, None] % 64 == pp[None, :] % 64).astype(np.float32)
    shared["fold4"] = f4
    shared["onesrow"] = np.ones((1, 128), np.float32)
    shared["fold2"] = f2
    shared = {k: np.ascontiguousarray(v, np.float32) for k, v in shared.items()}

    in_maps = []
    for c in range(N_CORES):
        p = pts[c * J:(c + 1) * J]          # [J, N, 2]
        f = feat[c * J:(c + 1) * J]          # [J, N, F]
        sq = (p * p).sum(-1)                 # [J, N]
        ptsA_ = np.stack([p[:, :, 0], p[:, :, 1], sq, np.ones_like(sq)], 0)
        ptsB_ = np.stack([-2 * p[:, :, 0], -2 * p[:, :, 1], np.ones_like(sq), sq], 0)
        featC_ = np.transpose(f, (2, 0, 1))  # [F, J, N]
        m = dict(shared)
        m["ptsA"] = np.ascontiguousarray(ptsA_, np.float32)
        m["ptsB"] = np.ascontiguousarray(ptsB_, np.float32)
        m["featC"] = np.ascontiguousarray(featC_, np.float32)
        in_maps.append(m)

    import time as _t
    _t0 = _t.time()
    res = run_bass_kernel_spmd(nc, in_maps, core_ids=list(range(N_CORES)))
    _t1 = _t.time()
    import os
    if os.environ.get("KERNEL_TRACE", "0") == "1":
        print(f"HW exec time: {int((_t1 - _t0) * 1e9)} ns (wall of spmd execute)")
    outs = [res.results[c]["out"].reshape(J) for c in range(N_CORES)]
    return np.concatenate(outs).reshape(B, 1).astype(np.float32)



# revision 6
# speedup vs baseline: 8.9276x; 8.9276x over previous
import numpy as np

# DGCNN (2x DynamicEdgeConv + global max pool + MLP head) on 8 NeuronCores.
# Data-parallel over jets (512 -> 64/core); BN batch statistics exact via
# 3 tiny AllReduces. v2: packed inputs (2 DRAM tensors), on-device constant
# assembly, matmul-based kNN scores, DRAM-bounced batched index wraps,
# reduce+matmul neighbor means.

N_CORES = 8
B, N, F = 512, 128, 16
J = B // N_CORES          # 64 jets per core
K = 20                    # neighbors used
NG1 = 4                   # conv1 jet-stack (4 x 32ch)
NG2 = 2                   # conv2 jet-stack (2 x 64ch)
G1 = J // NG1             # 16 groups conv1
G2 = J // NG2             # 32 groups conv2
E = 2560                  # K*N edges per jet
USE_ALLREDUCE = True
EPS = 1e-5
NEG = -1.0e30
WC = 107                  # wpack columns (f32 small weights)
MC = 264                  # mh16 columns (fp16 head weights)
OFF_MH = 132096           # fp16-unit offsets into the packed input tensor
OFF_WP = 165888
OFF_PT = 193280
PK_LEN = 242432


def _build_nc():
    import concourse.bass as bass
    import concourse.mybir as mybir
    import concourse.tile as tile
    from concourse import bacc

    fp32 = mybir.dt.float32
    fp16 = mybir.dt.float16
    i16 = mybir.dt.int16
    u32 = mybir.dt.uint32
    AF = mybir.ActivationFunctionType
    OP = mybir.AluOpType
    AX = mybir.AxisListType

    nc = bacc.Bacc(None)

    pk = nc.dram_tensor("pk", [1, PK_LEN], fp16, kind="ExternalInput")
    out_t = nc.dram_tensor("out", [1, J], fp32, kind="ExternalOutput")

    idxd1 = nc.dram_tensor("idxd1", [J, N, K], i16)
    idxd2 = nc.dram_tensor("idxd2", [J, N, K], i16)
    cc_in = [nc.dram_tensor(f"cc_in{i}", [128, 4], fp32) for i in range(3)]
    cc_out = [nc.dram_tensor(f"cc_out{i}", [128, 4], fp32) for i in range(3)]

    def dview(t, off, dims):
        base = t[:, :] if len(t.shape) == 2 else t[:, :, :]
        return bass.AP(tensor=base.tensor, offset=off, ap=dims)

    def sview(ap, extra_off, dims):
        # strided view of an SBUF AP: keep partition dim, custom free dims
        return bass.AP(tensor=ap.tensor, offset=ap.offset + extra_off,
                       ap=[ap.ap[0]] + dims)

    with tile.TileContext(nc) as tc:
        with (
            tc.tile_pool(name="persist", bufs=1) as P,
            tc.tile_pool(name="work", bufs=2) as W,
            tc.tile_pool(name="blk", bufs=2) as BK,
            tc.tile_pool(name="small", bufs=4) as S,
            tc.tile_pool(name="gatp", bufs=1) as WG,
            tc.tile_pool(name="stats", bufs=1) as ST,
            tc.tile_pool(name="psum", bufs=3, space="PSUM") as PS,
            tc.tile_pool(name="psum2", bufs=2, space="PSUM") as PS2,
            tc.tile_pool(name="psum3", bufs=3, space="PSUM") as PS3,
        ):
            # ---- unpack weights from the packed input ----
            def wload(r0, c0, rr, cc, tag):
                # f32 block stored as raw bytes in the fp16 container
                sb = P.tile([rr, cc], fp32, tag=tag)
                v = dview(pk, OFF_WP + (r0 * WC + c0) * 2,
                          [[WC * 2, rr], [1, cc * 2]]).bitcast(fp32)
                nc.sync.dma_start(out=sb, in_=v)
                return sb

            w2pd_s = wload(0, 0, 64, 64, "w2pd")
            w2_s = wload(0, 64, 32, 32, "w2")
            w3_s = wload(32, 64, 32, 32, "w3")
            eye32_s = wload(64, 64, 32, 32, "eye32")
            biasb = wload(0, 96, 128, 11, "biasb")
            mh2_s = P.tile([128, 128], fp16, tag="mh2")
            mh1_s = P.tile([64, 128], fp16, tag="mh1")
            mh3_s = P.tile([128, 8], fp16, tag="mh3")
            nc.sync.dma_start(out=mh2_s, in_=dview(pk, OFF_MH, [[MC, 128], [1, 128]]))
            nc.sync.dma_start(out=mh1_s, in_=dview(pk, OFF_MH + 128, [[MC, 64], [1, 128]]))
            nc.sync.dma_start(out=mh3_s, in_=dview(pk, OFF_MH + 256, [[MC, 128], [1, 8]]))
            w1ah = P.tile([16, 32], fp16, tag="w1ah")
            w1bh = P.tile([16, 32], fp16, tag="w1bh")
            nc.sync.dma_start(out=w1ah, in_=dview(pk, J * N, [[J * N + 64, 16], [1, 32]]))
            nc.sync.dma_start(out=w1bh, in_=dview(pk, J * N + 32, [[J * N + 64, 16], [1, 32]]))
            g1r_s = biasb[:, 0:1]
            be1r_s = biasb[:, 1:2]
            g2r_s = biasb[:, 2:3]
            be2r_s = biasb[:, 3:4]
            b3r_s = biasb[:, 4:5]
            g3r_s = biasb[:, 5:6]
            be3r_s = biasb[:, 6:7]
            b2pr_s = biasb[:, 7:8]
            mb1_s = biasb[:, 8:9]
            mb2_s = biasb[:, 9:10]
            mb3_s = biasb[:, 10:11]

            # replicated conv2-L1 weights at all 4 bands
            W2PA4 = P.tile([128, 64], fp32, tag="W2PA4")
            W2PB4 = P.tile([128, 64], fp32, tag="W2PB4")
            for k in range(4):
                nc.sync.dma_start(
                    out=W2PA4[k * 32:(k + 1) * 32, :],
                    in_=dview(pk, OFF_WP + (64 * WC) * 2,
                              [[WC * 2, 32], [1, 128]]).bitcast(fp32))
                nc.sync.dma_start(
                    out=W2PB4[k * 32:(k + 1) * 32, :],
                    in_=dview(pk, OFF_WP + (96 * WC) * 2,
                              [[WC * 2, 32], [1, 128]]).bitcast(fp32))

            # ---- on-device constant assembly ----
            w2bd_s = P.tile([128, 128], fp32, tag="w2bd")
            w3bd_s = P.tile([128, 128], fp32, tag="w3bd")
            w2pbd_s = P.tile([128, 128], fp32, tag="w2pbd")
            nc.vector.memset(w2bd_s, 0.0)
            nc.vector.memset(w3bd_s, 0.0)
            nc.vector.memset(w2pbd_s, 0.0)
            for k in range(4):
                nc.sync.dma_start(
                    out=w2bd_s[k * 32:(k + 1) * 32, k * 32:(k + 1) * 32], in_=w2_s)
                nc.sync.dma_start(
                    out=w3bd_s[k * 32:(k + 1) * 32, k * 32:(k + 1) * 32], in_=w3_s)
            for k in range(2):
                nc.sync.dma_start(
                    out=w2pbd_s[k * 64:(k + 1) * 64, k * 64:(k + 1) * 64],
                    in_=w2pd_s)

            fold4_s = P.tile([128, 128], fp32, tag="fold4")
            fold2_s = P.tile([128, 128], fp32, tag="fold2")
            nc.vector.memset(fold4_s, 0.0)
            nc.vector.memset(fold2_s, 0.0)
            for bi in range(4):
                for bj in range(4):
                    nc.sync.dma_start(
                        out=fold4_s[bi * 32:(bi + 1) * 32, bj * 32:(bj + 1) * 32],
                        in_=eye32_s)
            for bi in range(2):
                for bj in range(2):
                    for a in range(2):
                        nc.sync.dma_start(
                            out=fold2_s[bi * 64 + a * 32:bi * 64 + (a + 1) * 32,
                                        bj * 64 + a * 32:bj * 64 + (a + 1) * 32],
                            in_=eye32_s)

            blk4s = P.tile([128, 128], fp32, tag="blk4s")
            nc.vector.memset(blk4s, 0.0)
            for k in range(4):
                nc.vector.memset(blk4s[k * 32:(k + 1) * 32, k * 32:k * 32 + 1], 1.0)
            ONES = P.tile([128, 128], fp32, tag="ONES")
            nc.vector.memset(ONES, 1.0)
            epsap = P.tile([128, 1], fp32, tag="epsap")
            nc.vector.memset(epsap, EPS)

            # ---- persistent intermediates ----
            IdxBig = P.tile([128, J, 24], u32, tag="IdxBig")
            Idx16 = P.tile([128, J, 20], i16, tag="Idx16")
            IdxW1 = P.tile([128, G1, 160], i16, tag="IdxW1")
            IdxW2 = P.tile([128, G2, 160], i16, tag="IdxW2")
            Bm1_all = P.tile([128, G1, N], fp32, tag="Bm1_all")
            SH1_all = P.tile([128, G1, N], fp32, tag="SH1_all")
            B2_all = P.tile([128, G2, N], fp32, tag="B2_all")
            SH2_all = P.tile([128, G2, N], fp32, tag="SH2_all")
            x1_all = P.tile([128, G1, N], fp32, tag="x1_all")
            pooled = P.tile([128, G2], fp32, tag="pooled")

            def topk_jet(score_ps, g, sgn):
                sc = W.tile([128, N], fp32, tag="score")
                nc.scalar.activation(out=sc, in_=score_ps, func=AF.Copy, scale=sgn)
                for r in range(3):
                    m8 = S.tile([128, 8], fp32, tag="m8")
                    nc.vector.max(out=m8, in_=sc)
                    nc.vector.max_index(
                        out=IdxBig[:, g, r * 8:(r + 1) * 8], in_max=m8, in_values=sc)
                    if r < 2:
                        nc.vector.match_replace(
                            out=sc, in_to_replace=m8, in_values=sc, imm_value=NEG)

            # ---- conv1 kNN (8-jet blocks) ----
            # psD = 4*xi.xj - 2*|xj|^2 = 2*(-D + |xi|^2) -> max per row = nearest
            for blk in range(J // 8):
                PXYb = BK.tile([2, 8, N], fp32, tag="pxyb")
                PM2 = BK.tile([2, 8, N], fp32, tag="pm2")
                sqn = BK.tile([1, 8, N], fp32, tag="sqn")
                nc.sync.dma_start(
                    out=PXYb, in_=dview(pk, OFF_PT + blk * 6144,
                                        [[2048, 2], [1, 2048]]).bitcast(fp32))
                nc.sync.dma_start(
                    out=sqn, in_=dview(pk, OFF_PT + blk * 6144 + 4096,
                                       [[2048, 1], [1, 2048]]).bitcast(fp32))
                nc.scalar.activation(out=PM2, in_=PXYb, func=AF.Copy, scale=-2.0)
                for j in range(8):
                    g = blk * 8 + j
                    psD = PS.tile([128, N], fp32, tag="ps")
                    nc.tensor.matmul(psD, PM2[:, j, :], PM2[:, j, :],
                                     start=True, stop=False)
                    nc.tensor.matmul(psD, ONES[0:1, :], sqn[0:1, j, :],
                                     start=False, stop=True)
                    topk_jet(psD, g, 1.0)

            # cast idx and bounce through DRAM in jet-major layout
            def idx_to_dram(idxd):
                nc.vector.tensor_copy(out=Idx16, in_=IdxBig[:, :, 1:21])
                dst = dview(idxd, 0, [[K, N], [N * K, J], [1, K]])
                nc.sync.dma_start(out=dst, in_=Idx16[:, :, :])

            idx_to_dram(idxd1)
            for grp in range(G1):
                for k in range(NG1):
                    src = dview(idxd1, (NG1 * grp + k) * N * K,
                                [[0, 2], [8 * K, 16], [K, 8], [1, K]])
                    nc.sync.dma_start(
                        out=IdxW1[k * 32:(k + 1) * 32, grp, :], in_=src)

            # ---- conv1 L1 (batched over 4-jet strided chunks) ----
            FT = P.tile([16, J, N], fp16, tag="bigA")
            nc.sync.dma_start(out=FT, in_=dview(pk, 0, [[J * N + 64, 16], [1, J * N]]))
            for k in range(NG1):
                for c in range(4):
                    ftv = FT[:, 16 * c + k, :]
                    rhs = sview(ftv, 0, [[NG1 * N, 4], [1, N]])
                    psA = PS3.tile([128, 512], fp32, tag="psh")
                    psB = PS3.tile([128, 512], fp32, tag="psh")
                    nc.tensor.matmul(psA[k * 32:(k + 1) * 32, :], w1ah, rhs,
                                     start=True, stop=True, tile_position=(0, k * 32))
                    nc.tensor.matmul(psB[k * 32:(k + 1) * 32, :], w1bh, rhs,
                                     start=True, stop=True, tile_position=(0, k * 32))
                    nc.scalar.copy(out=Bm1_all[k * 32:(k + 1) * 32, 4 * c:4 * c + 4, :],
                                   in_=psB[k * 32:(k + 1) * 32, :])
                    nc.vector.tensor_sub(
                        out=SH1_all[k * 32:(k + 1) * 32, 4 * c:4 * c + 4, :],
                        in0=psA[k * 32:(k + 1) * 32, :],
                        in1=Bm1_all[k * 32:(k + 1) * 32, 4 * c:4 * c + 4, :])

            def edge_pass(src, SH, IdxW, ngrp, mode, s1=None, t1=None,
                          s2=None, t2=None, stats_t=None, wfold=None,
                          xout=None, b3ap=None):
                for grp in range(ngrp):
                    gat = WG.tile([128, E], fp32, tag="gat")
                    nc.gpsimd.ap_gather(
                        out_ap=gat[:, :], in_ap=src[:, grp, :],
                        idxs_ap=IdxW[:, grp, :],
                        channels=128, num_elems=N, d=1, num_idxs=E)
                    shv = SH[:, grp, :]
                    sh_b = sview(shv, 0, [[1, 8], [0, 20], [8, 16]])
                    g4 = gat.rearrange("p (a b q) -> p a b q", b=20, q=16)
                    nc.vector.tensor_add(out=g4, in0=g4, in1=sh_b)
                    if mode == "stats1":
                        for c in range(5):
                            nc.vector.bn_stats(
                                out=stats_t[:, grp * 5 + c, :],
                                in_=gat[:, c * 512:(c + 1) * 512])
                        continue
                    r1 = P.tile([128, E], fp32, tag="r1")
                    nc.scalar.activation(out=r1, in_=gat, func=AF.Relu,
                                         bias=t1, scale=s1)

                    def mean_fold(rr, psx):
                        # sum over neighbor dim b (strided view, b innermost)
                        rv = rr[:, :]
                        red = W.tile([128, N], fp32, tag="red")
                        rin = sview(rv, 0, [[320, 8], [1, 16], [16, 20]])
                        rout = sview(red[:, :], 0, [[16, 8], [1, 16]])
                        nc.vector.tensor_reduce(out=rout, in_=rin,
                                                axis=AX.X, op=OP.add)
                        nc.tensor.matmul(psx, wfold, red, start=True, stop=True)

                    if mode == "final2":
                        psx = PS2.tile([128, N], fp32, tag="psx")
                        mean_fold(r1, psx)
                        pm = S.tile([128, 1], fp32, tag="pm")
                        nc.vector.tensor_reduce(out=pm, in_=psx,
                                                axis=AX.X, op=OP.max)
                        nc.vector.tensor_scalar(out=pooled[:, grp:grp + 1], in0=pm,
                                                scalar1=1.0 / K, scalar2=b3ap,
                                                op0=OP.mult, op1=OP.add)
                        continue
                    r2 = None
                    if mode == "final1":
                        r2 = P.tile([128, E], fp32, tag="r2")
                    for c in range(5):
                        p = PS3.tile([128, 512], fp32, tag="psh")
                        nc.tensor.matmul(p, w2bd_s, r1[:, c * 512:(c + 1) * 512],
                                         start=True, stop=True)
                        if mode == "stats2":
                            nc.vector.bn_stats(out=stats_t[:, grp * 5 + c, :],
                                               in_=p)
                        else:
                            nc.scalar.activation(out=r2[:, c * 512:(c + 1) * 512],
                                                 in_=p, func=AF.Relu,
                                                 bias=t2, scale=s2)
                    if mode == "stats2":
                        continue
                    psx = PS2.tile([128, N], fp32, tag="psx")
                    mean_fold(r2, psx)
                    nc.vector.tensor_scalar(out=xout[:, grp, :], in0=psx,
                                            scalar1=1.0 / K, scalar2=b3ap,
                                            op0=OP.mult, op1=OP.add)

            def bn_param(stats_t, nchunk, foldm, gam, bet, cci, cco, nunits):
                mv = S.tile([128, 2], fp32, tag="mv")
                if nchunk <= 80:
                    nc.vector.bn_aggr(out=mv, in_=stats_t[:, 0:nchunk, :])
                else:
                    h = nchunk // 2
                    mv1 = S.tile([128, 2], fp32, tag="mv1")
                    mv2 = S.tile([128, 2], fp32, tag="mv2")
                    nc.vector.bn_aggr(out=mv1, in_=stats_t[:, 0:h, :])
                    nc.vector.bn_aggr(out=mv2, in_=stats_t[:, h:nchunk, :])
                    d = S.tile([128, 1], fp32, tag="mvd")
                    nc.vector.tensor_sub(out=d, in0=mv1[:, 0:1], in1=mv2[:, 0:1])
                    nc.vector.tensor_scalar_mul(out=d, in0=d, scalar1=0.5)
                    nc.vector.tensor_mul(out=d, in0=d, in1=d)
                    nc.vector.tensor_add(out=mv[:, 0:1], in0=mv1[:, 0:1], in1=mv2[:, 0:1])
                    nc.vector.tensor_scalar_mul(out=mv[:, 0:1], in0=mv[:, 0:1], scalar1=0.5)
                    nc.vector.tensor_add(out=mv[:, 1:2], in0=mv1[:, 1:2], in1=mv2[:, 1:2])
                    nc.vector.tensor_scalar(out=mv[:, 1:2], in0=mv[:, 1:2],
                                            scalar1=0.5, scalar2=None, op0=OP.mult)
                    nc.vector.tensor_add(out=mv[:, 1:2], in0=mv[:, 1:2], in1=d)
                pay = S.tile([128, 4], fp32, tag="pay")
                nc.vector.tensor_copy(out=pay[:, 0:2], in_=mv)
                nc.vector.tensor_mul(out=pay[:, 2:3], in0=mv[:, 0:1], in1=mv[:, 0:1])
                nc.vector.memset(pay[:, 3:4], 0.0)
                if USE_ALLREDUCE:
                    nc.gpsimd.dma_start(out=cci[:, :], in_=pay)
                    nc.gpsimd.collective_compute(
                        "AllReduce", OP.add,
                        replica_groups=[list(range(N_CORES))],
                        ins=[cci[:, :]], outs=[cco[:, :]])
                    arr = S.tile([128, 4], fp32, tag="arr")
                    nc.gpsimd.dma_start(out=arr, in_=cco[:, :])
                else:
                    arr = pay
                psf = PS.tile([128, 4], fp32, tag="ps")
                nc.tensor.matmul(psf, foldm, arr, start=True, stop=True)
                mg = S.tile([128, 1], fp32, tag="mg")
                vg = S.tile([128, 1], fp32, tag="vg")
                nc.vector.tensor_scalar_mul(out=mg, in0=psf[:, 0:1], scalar1=1.0 / nunits)
                m2g = S.tile([128, 1], fp32, tag="m2g")
                nc.vector.tensor_scalar_mul(out=m2g, in0=psf[:, 2:3], scalar1=1.0 / nunits)
                nc.vector.tensor_scalar_mul(out=vg, in0=psf[:, 1:2], scalar1=1.0 / nunits)
                nc.vector.tensor_add(out=vg, in0=vg, in1=m2g)
                mm = S.tile([128, 1], fp32, tag="mm")
                nc.vector.tensor_mul(out=mm, in0=mg, in1=mg)
                nc.vector.tensor_sub(out=vg, in0=vg, in1=mm)
                sd = S.tile([128, 1], fp32, tag="sd")
                nc.scalar.activation(out=sd, in_=vg, func=AF.Sqrt, bias=epsap, scale=1.0)
                ri = S.tile([128, 1], fp32, tag="ri")
                nc.vector.reciprocal(out=ri, in_=sd)
                s = P.tile([128, 1], fp32, tag=f"bn_s_{cci.name}")
                t = P.tile([128, 1], fp32, tag=f"bn_t_{cci.name}")
                nc.vector.tensor_mul(out=s, in0=gam, in1=ri)
                nc.vector.tensor_mul(out=t, in0=mg, in1=s)
                nc.vector.tensor_sub(out=t, in0=bet, in1=t)
                return s, t

            # ---- conv1 stats + passes ----
            statsA = ST.tile([128, G2 * 5, 6], fp32, tag="stats")
            stats1 = statsA[:, 0:G1 * 5, :]
            edge_pass(Bm1_all, SH1_all, IdxW1, G1, "stats1", stats_t=stats1)
            s1, t1 = bn_param(stats1, G1 * 5, fold4_s, g1r_s, be1r_s,
                              cc_in[0], cc_out[0],
                              4 * N_CORES if USE_ALLREDUCE else 4)
            statsB = ST.tile([128, G2 * 5, 6], fp32, tag="stats")
            stats2 = statsB[:, 0:G1 * 5, :]
            edge_pass(Bm1_all, SH1_all, IdxW1, G1, "stats2", s1=s1, t1=t1,
                      stats_t=stats2)
            s2, t2 = bn_param(stats2, G1 * 5, fold4_s, g2r_s, be2r_s,
                              cc_in[1], cc_out[1],
                              4 * N_CORES if USE_ALLREDUCE else 4)
            edge_pass(Bm1_all, SH1_all, IdxW1, G1, "final1", s1=s1, t1=t1,
                      s2=s2, t2=t2, wfold=w3bd_s, xout=x1_all, b3ap=b3r_s)

            # ---- conv2 prep: -2x and banded squared norms ----
            X2 = P.tile([128, G1, N], fp32, tag="bigA")
            sqx = P.tile([128, G1, N], fp32, tag="sqx")
            sqn_s = P.tile([128, G1, N], fp32, tag="sqn_s")
            nc.scalar.activation(out=X2, in_=x1_all, func=AF.Copy, scale=-2.0)
            nc.vector.tensor_mul(out=sqx, in0=x1_all, in1=x1_all)
            for c in range(4):
                pss = PS3.tile([128, 512], fp32, tag="psh")
                nc.tensor.matmul(pss, blk4s, sqx[:, 4 * c:4 * c + 4, :],
                                 start=True, stop=True)
                nc.scalar.copy(out=sqn_s[:, 4 * c:4 * c + 4, :], in_=pss)

            # ---- conv2 kNN ----
            for g in range(J):
                k = g % NG1
                grp = g // NG1
                psD = PS.tile([128, N], fp32, tag="ps")
                nc.tensor.matmul(psD, X2[k * 32:(k + 1) * 32, grp, :],
                                 x1_all[k * 32:(k + 1) * 32, grp, :],
                                 start=True, stop=False,
                                 tile_position=(k * 32, 0))
                nc.tensor.matmul(psD, ONES[k * 32:k * 32 + 1, :],
                                 sqn_s[k * 32:k * 32 + 1, grp, :],
                                 start=False, stop=True,
                                 tile_position=(k * 32, 0))
                topk_jet(psD, g, -1.0)

            idx_to_dram(idxd2)
            for grp2 in range(G2):
                for k2 in range(NG2):
                    src = dview(idxd2, (NG2 * grp2 + k2) * N * K,
                                [[0, 4], [8 * K, 16], [K, 8], [1, K]])
                    nc.sync.dma_start(
                        out=IdxW2[k2 * 64:(k2 + 1) * 64, grp2, :], in_=src)

            # ---- conv2 L1 (batched) ----
            for k1 in range(4):
                b2 = (k1 % 2) * 64
                go = k1 // 2
                for c in range(4):
                    psA = PS3.tile([128, 512], fp32, tag="psh")
                    psB = PS3.tile([128, 512], fp32, tag="psh")
                    rhs = x1_all[k1 * 32:(k1 + 1) * 32, 4 * c:4 * c + 4, :]
                    nc.tensor.matmul(psA[b2:b2 + 64, :],
                                     W2PA4[k1 * 32:(k1 + 1) * 32, :], rhs,
                                     start=True, stop=True,
                                     tile_position=(k1 * 32, b2))
                    nc.tensor.matmul(psB[b2:b2 + 64, :],
                                     W2PB4[k1 * 32:(k1 + 1) * 32, :], rhs,
                                     start=True, stop=True,
                                     tile_position=(k1 * 32, b2))
                    b2v = B2_all[b2:b2 + 64, 0, :]
                    dstB = sview(b2v, (8 * c + go) * N, [[2 * N, 4], [1, N]])
                    s2v = SH2_all[b2:b2 + 64, 0, :]
                    dstS = sview(s2v, (8 * c + go) * N, [[2 * N, 4], [1, N]])
                    nc.scalar.copy(out=dstB, in_=psB[b2:b2 + 64, :])
                    nc.vector.tensor_sub(out=dstS, in0=psA[b2:b2 + 64, :], in1=dstB)

            # ---- conv2 stats + final ----
            stats3 = ST.tile([128, G2 * 5, 6], fp32, tag="stats")
            edge_pass(B2_all, SH2_all, IdxW2, G2, "stats1", stats_t=stats3)
            s3, t3 = bn_param(stats3, G2 * 5, fold2_s, g3r_s, be3r_s,
                              cc_in[2], cc_out[2],
                              2 * N_CORES if USE_ALLREDUCE else 2)
            edge_pass(B2_all, SH2_all, IdxW2, G2, "final2", s1=s3, t1=t3,
                      wfold=w2pbd_s, b3ap=b2pr_s)

            # ---- head (fp16 weights/activations) ----
            pooledh = W.tile([128, G2], fp16, tag="pooledh")
            nc.vector.tensor_copy(out=pooledh, in_=pooled)
            Gh = P.tile([64, J], fp16, tag="Gh")
            gh_v = Gh.rearrange("p (g s) -> p g s", s=2)
            nc.sync.dma_start(out=gh_v[:, :, 0], in_=pooledh[0:64, :])
            nc.sync.dma_start(out=gh_v[:, :, 1], in_=pooledh[64:128, :])
            ps1 = PS.tile([128, J], fp32, tag="ps")
            nc.tensor.matmul(ps1, mh1_s, Gh, start=True, stop=True)
            hh1 = W.tile([128, J], fp16, tag="hh1")
            nc.scalar.activation(out=hh1, in_=ps1, func=AF.Relu, bias=mb1_s, scale=1.0)
            ps2 = PS.tile([128, J], fp32, tag="ps")
            nc.tensor.matmul(ps2, mh2_s, hh1, start=True, stop=True)
            hh2 = W.tile([128, J], fp16, tag="hh2")
            nc.scalar.activation(out=hh2, in_=ps2, func=AF.Relu, bias=mb2_s, scale=1.0)
            ps3 = PS.tile([8, J], fp32, tag="ps")
            nc.tensor.matmul(ps3, mh3_s, hh2, start=True, stop=True)
            ov = W.tile([1, J], fp32, tag="ov")
            nc.vector.tensor_scalar(out=ov, in0=ps3[0:1, :], scalar1=mb3_s[0:1, 0:1],
                                    scalar2=None, op0=OP.add)
            nc.sync.dma_start(out=out_t[:, :], in_=ov)

    nc.finalize()
    return nc


_NC_CACHE = None
_CACHE_SET = False
LAST_EXEC_NS = None


def _enable_jax_cache():
    global _CACHE_SET
    if _CACHE_SET:
        return
    import jax
    jax.config.update("jax_compilation_cache_dir", "/tmp/bass_jax_cache_v2")
    jax.config.update("jax_persistent_cache_min_compile_time_secs", 0.0)
    jax.config.update("jax_persistent_cache_min_entry_size_bytes", 0)
    _CACHE_SET = True


def _pack_weights(i):
    wp = np.zeros((128, WC), np.float32)
    wp[0:64, 0:64] = i["c2_w2"]
    wp[64:96, 0:64] = i["c2_w1"][:32]
    wp[96:128, 0:64] = i["c2_w1"][32:]
    wp[0:32, 64:96] = i["c1_w2"]
    wp[32:64, 64:96] = i["c1_w3"]
    wp[64:96, 64:96] = np.eye(32, dtype=np.float32)
    wp[0:128, 96] = np.tile(i["c1_g1"], 4)
    wp[0:128, 97] = np.tile(i["c1_be1"], 4)
    wp[0:128, 98] = np.tile(i["c1_g2"], 4)
    wp[0:128, 99] = np.tile(i["c1_be2"], 4)
    wp[0:128, 100] = np.tile(i["c1_b3"], 4)
    wp[0:128, 101] = np.tile(i["c2_g1"], 2)
    wp[0:128, 102] = np.tile(i["c2_be1"], 2)
    wp[0:128, 103] = np.tile(i["c2_b2"], 2)
    wp[0:128, 104] = i["m_b1"]
    wp[0:128, 105] = i["m_b2"]
    wp[0, 106] = i["m_b3"][0]
    mh = np.zeros((128, MC), np.float16)
    mh[0:128, 0:128] = i["m_w2"].astype(np.float16)
    mh[0:64, 128:256] = i["m_w1"].astype(np.float16)
    mh[0:128, 256:257] = i["m_w3"].astype(np.float16)
    return wp, mh


def kernel(**inputs) -> np.ndarray:
    global _NC_CACHE, LAST_EXEC_NS
    _enable_jax_cache()
    from concourse.bass_utils import run_bass_kernel_spmd

    if _NC_CACHE is None:
        _NC_CACHE = _build_nc()
        # the module is immutable after finalize(); memoize its JSON so the
        # per-call jit lowering doesn't re-serialize 3MB of BIR every time
        _json = _NC_CACHE.to_json_bytes()
        _NC_CACHE.to_json_bytes = lambda _j=_json: _j
    nc = _NC_CACHE

    pts = inputs["points"].astype(np.float32)
    feat = inputs["features"].astype(np.float32)
    wp, mh = _pack_weights({k: np.asarray(v, np.float32) for k, v in inputs.items()
                            if k not in ("points", "features")})

    w1 = np.asarray(inputs["c1_w1"], np.float32)
    wh = np.concatenate([w1[:16], w1[16:]], axis=1).astype(np.float16)
    mh_flat = mh.reshape(-1)
    wp_bits = wp.reshape(-1).view(np.float16)
    # vectorized packing across all cores at once
    ft16 = feat.transpose(2, 0, 1).astype(np.float16)      # [16, B, N]
    d_all = np.empty((N_CORES, 3, J, N), np.float32)
    d_all[:, 0] = pts[:, :, 0].reshape(N_CORES, J, N)
    d_all[:, 1] = pts[:, :, 1].reshape(N_CORES, J, N)
    d_all[:, 2] = -2.0 * (pts[:, :, 0] ** 2 + pts[:, :, 1] ** 2).reshape(N_CORES, J, N)
    # per-block layout: (core, blk, row{x,y,sqn}, jet, node)
    dblk_all = np.ascontiguousarray(
        d_all.reshape(N_CORES, 3, 8, 8, N).transpose(0, 2, 1, 3, 4))
    dbits = dblk_all.reshape(N_CORES, -1).view(np.float16)
    in_maps = []
    for c in range(N_CORES):
        pkv = np.empty(PK_LEN, np.float16)
        fc2 = pkv[0:OFF_MH].reshape(16, J * N + 64)
        fc2[:, 0:J * N] = ft16[:, c * J:(c + 1) * J, :].reshape(16, J * N)
        fc2[:, J * N:] = wh
        pkv[OFF_MH:OFF_WP] = mh_flat
        pkv[OFF_WP:OFF_PT] = wp_bits
        pkv[OFF_PT:] = dbits[c]
        in_maps.append({"pk": pkv.reshape(1, PK_LEN)})

    import time as _t
    _t0 = _t.time()
    try:
        res = run_bass_kernel_spmd(nc, in_maps, core_ids=list(range(N_CORES)))
    except Exception:
        # transient device hiccup (e.g. NRT_EXEC_UNIT_UNRECOVERABLE): retry once
        _t0 = _t.time()
        res = run_bass_kernel_spmd(nc, in_maps, core_ids=list(range(N_CORES)))
    _t1 = _t.time()
    LAST_EXEC_NS = int((_t1 - _t0) * 1e9)
    import os
    if os.environ.get("KERNEL_TRACE", "0") == "1":
        print(f"HW exec time: {LAST_EXEC_NS} ns (wall of spmd execute)")
    outs = [res.results[c]["out"].reshape(J) for c in range(N_CORES)]
    return np.concatenate(outs).reshape(B, 1).astype(np.float32)


# revision 7
# speedup vs baseline: 9.1428x; 1.0241x over previous
import numpy as np

# DGCNN (2x DynamicEdgeConv + global max pool + MLP head) on 8 NeuronCores.
# Data-parallel over jets (512 -> 64/core); BN batch statistics exact via
# 3 tiny AllReduces. v2: packed inputs (2 DRAM tensors), on-device constant
# assembly, matmul-based kNN scores, DRAM-bounced batched index wraps,
# reduce+matmul neighbor means.

N_CORES = 8
B, N, F = 512, 128, 16
J = B // N_CORES          # 64 jets per core
K = 20                    # neighbors used
NG1 = 4                   # conv1 jet-stack (4 x 32ch)
NG2 = 2                   # conv2 jet-stack (2 x 64ch)
G1 = J // NG1             # 16 groups conv1
G2 = J // NG2             # 32 groups conv2
E = 2560                  # K*N edges per jet
USE_ALLREDUCE = True
EPS = 1e-5
NEG = -1.0e30
WC = 107                  # wpack columns (f32 small weights)
MC = 264                  # mh16 columns (fp16 head weights)
OFF_MH = 132096           # fp16-unit offsets into the packed input tensor
OFF_WP = 165888
OFF_PT = 193280
PK_LEN = 242432


def _build_nc():
    import concourse.bass as bass
    import concourse.mybir as mybir
    import concourse.tile as tile
    from concourse import bacc

    fp32 = mybir.dt.float32
    fp16 = mybir.dt.float16
    i16 = mybir.dt.int16
    u32 = mybir.dt.uint32
    AF = mybir.ActivationFunctionType
    OP = mybir.AluOpType
    AX = mybir.AxisListType

    nc = bacc.Bacc(None)

    pk = nc.dram_tensor("pk", [1, PK_LEN], fp16, kind="ExternalInput")
    out_t = nc.dram_tensor("out", [1, J], fp32, kind="ExternalOutput")

    idxd1 = nc.dram_tensor("idxd1", [J, N, K], i16)
    idxd2 = nc.dram_tensor("idxd2", [J, N, K], i16)
    cc_in = [nc.dram_tensor(f"cc_in{i}", [128, 4], fp32) for i in range(3)]
    cc_out = [nc.dram_tensor(f"cc_out{i}", [128, 4], fp32) for i in range(3)]

    def dview(t, off, dims):
        base = t[:, :] if len(t.shape) == 2 else t[:, :, :]
        return bass.AP(tensor=base.tensor, offset=off, ap=dims)

    def sview(ap, extra_off, dims):
        # strided view of an SBUF AP: keep partition dim, custom free dims
        return bass.AP(tensor=ap.tensor, offset=ap.offset + extra_off,
                       ap=[ap.ap[0]] + dims)

    with tile.TileContext(nc) as tc:
        with (
            tc.tile_pool(name="persist", bufs=1) as P,
            tc.tile_pool(name="work", bufs=2) as W,
            tc.tile_pool(name="blk", bufs=2) as BK,
            tc.tile_pool(name="small", bufs=4) as S,
            tc.tile_pool(name="gatp", bufs=1) as WG,
            tc.tile_pool(name="stats", bufs=1) as ST,
            tc.tile_pool(name="psum", bufs=3, space="PSUM") as PS,
            tc.tile_pool(name="psum2", bufs=2, space="PSUM") as PS2,
            tc.tile_pool(name="psum3", bufs=3, space="PSUM") as PS3,
        ):
            # ---- unpack weights from the packed input ----
            def wload(r0, c0, rr, cc, tag):
                # f32 block stored as raw bytes in the fp16 container
                sb = P.tile([rr, cc], fp32, tag=tag)
                v = dview(pk, OFF_WP + (r0 * WC + c0) * 2,
                          [[WC * 2, rr], [1, cc * 2]]).bitcast(fp32)
                nc.sync.dma_start(out=sb, in_=v)
                return sb

            w2pd_s = wload(0, 0, 64, 64, "w2pd")
            w2_s = wload(0, 64, 32, 32, "w2")
            w3_s = wload(32, 64, 32, 32, "w3")
            eye32_s = wload(64, 64, 32, 32, "eye32")
            biasb = wload(0, 96, 128, 11, "biasb")
            mh2_s = P.tile([128, 128], fp16, tag="mh2")
            mh1_s = P.tile([64, 128], fp16, tag="mh1")
            mh3_s = P.tile([128, 1], fp16, tag="mh3")
            nc.sync.dma_start(out=mh2_s, in_=dview(pk, OFF_MH, [[MC, 128], [1, 128]]))
            # mh1 stored transposed [128,64]; DMA un-transposes via strided view
            nc.sync.dma_start(out=mh1_s, in_=dview(pk, OFF_MH + 128,
                                                   [[1, 64], [MC, 128]]))
            nc.sync.dma_start(out=mh3_s, in_=dview(pk, OFF_MH + 192,
                                                   [[MC, 128], [1, 1]]))
            w1ah = P.tile([16, 32], fp16, tag="w1ah")
            w1bh = P.tile([16, 32], fp16, tag="w1bh")
            nc.sync.dma_start(out=w1ah, in_=dview(pk, J * N, [[J * N + 64, 16], [1, 32]]))
            nc.sync.dma_start(out=w1bh, in_=dview(pk, J * N + 32, [[J * N + 64, 16], [1, 32]]))
            g1r_s = biasb[:, 0:1]
            be1r_s = biasb[:, 1:2]
            g2r_s = biasb[:, 2:3]
            be2r_s = biasb[:, 3:4]
            b3r_s = biasb[:, 4:5]
            g3r_s = biasb[:, 5:6]
            be3r_s = biasb[:, 6:7]
            b2pr_s = biasb[:, 7:8]
            mb1_s = biasb[:, 8:9]
            mb2_s = biasb[:, 9:10]
            mb3_s = biasb[:, 10:11]

            # replicated conv2-L1 weights at all 4 bands
            W2PA4 = P.tile([128, 64], fp32, tag="W2PA4")
            W2PB4 = P.tile([128, 64], fp32, tag="W2PB4")
            for k in range(4):
                nc.sync.dma_start(
                    out=W2PA4[k * 32:(k + 1) * 32, :],
                    in_=dview(pk, OFF_WP + (64 * WC) * 2,
                              [[WC * 2, 32], [1, 128]]).bitcast(fp32))
                nc.sync.dma_start(
                    out=W2PB4[k * 32:(k + 1) * 32, :],
                    in_=dview(pk, OFF_WP + (96 * WC) * 2,
                              [[WC * 2, 32], [1, 128]]).bitcast(fp32))

            # ---- on-device constant assembly ----
            w2bd_s = P.tile([128, 128], fp32, tag="w2bd")
            w3bd_s = P.tile([128, 128], fp32, tag="w3bd")
            w2pbd_s = P.tile([128, 128], fp32, tag="w2pbd")
            nc.vector.memset(w2bd_s, 0.0)
            nc.vector.memset(w3bd_s, 0.0)
            nc.vector.memset(w2pbd_s, 0.0)
            for k in range(4):
                nc.sync.dma_start(
                    out=w2bd_s[k * 32:(k + 1) * 32, k * 32:(k + 1) * 32], in_=w2_s)
                nc.sync.dma_start(
                    out=w3bd_s[k * 32:(k + 1) * 32, k * 32:(k + 1) * 32], in_=w3_s)
            for k in range(2):
                nc.sync.dma_start(
                    out=w2pbd_s[k * 64:(k + 1) * 64, k * 64:(k + 1) * 64],
                    in_=w2pd_s)

            fold4_s = P.tile([128, 128], fp32, tag="fold4")
            fold2_s = P.tile([128, 128], fp32, tag="fold2")
            nc.vector.memset(fold4_s, 0.0)
            nc.vector.memset(fold2_s, 0.0)
            for bi in range(4):
                for bj in range(4):
                    nc.sync.dma_start(
                        out=fold4_s[bi * 32:(bi + 1) * 32, bj * 32:(bj + 1) * 32],
                        in_=eye32_s)
            for bi in range(2):
                for bj in range(2):
                    for a in range(2):
                        nc.sync.dma_start(
                            out=fold2_s[bi * 64 + a * 32:bi * 64 + (a + 1) * 32,
                                        bj * 64 + a * 32:bj * 64 + (a + 1) * 32],
                            in_=eye32_s)

            blk4s = P.tile([128, 128], fp32, tag="blk4s")
            nc.vector.memset(blk4s, 0.0)
            for k in range(4):
                nc.vector.memset(blk4s[k * 32:(k + 1) * 32, k * 32:k * 32 + 1], 1.0)
            ONES = P.tile([128, 128], fp32, tag="ONES")
            nc.vector.memset(ONES, 1.0)
            epsap = P.tile([128, 1], fp32, tag="epsap")
            nc.vector.memset(epsap, EPS)

            # ---- persistent intermediates ----
            IdxBig = P.tile([128, J, 24], u32, tag="IdxBig")
            Idx16 = P.tile([128, J, 20], i16, tag="Idx16")
            IdxW1 = P.tile([128, G1, 160], i16, tag="IdxW1")
            IdxW2 = P.tile([128, G2, 160], i16, tag="IdxW2")
            Bm1_all = P.tile([128, G1, N], fp32, tag="Bm1_all")
            SH1_all = P.tile([128, G1, N], fp32, tag="SH1_all")
            B2_all = P.tile([128, G2, N], fp32, tag="B2_all")
            SH2_all = P.tile([128, G2, N], fp32, tag="SH2_all")
            x1_all = P.tile([128, G1, N], fp32, tag="x1_all")
            pooled = P.tile([128, G2], fp32, tag="pooled")

            def topk_jet(score_ps, g, sgn):
                sc = W.tile([128, N], fp32, tag="score")
                nc.scalar.activation(out=sc, in_=score_ps, func=AF.Copy, scale=sgn)
                for r in range(3):
                    m8 = S.tile([128, 8], fp32, tag="m8")
                    nc.vector.max(out=m8, in_=sc)
                    nc.vector.max_index(
                        out=IdxBig[:, g, r * 8:(r + 1) * 8], in_max=m8, in_values=sc)
                    if r < 2:
                        nc.vector.match_replace(
                            out=sc, in_to_replace=m8, in_values=sc, imm_value=NEG)

            # ---- conv1 kNN (8-jet blocks) ----
            # psD = 4*xi.xj - 2*|xj|^2 = 2*(-D + |xi|^2) -> max per row = nearest
            for blk in range(J // 8):
                PXYb = BK.tile([2, 8, N], fp32, tag="pxyb")
                PM2 = BK.tile([2, 8, N], fp32, tag="pm2")
                sqn = BK.tile([1, 8, N], fp32, tag="sqn")
                nc.sync.dma_start(
                    out=PXYb, in_=dview(pk, OFF_PT + blk * 6144,
                                        [[2048, 2], [1, 2048]]).bitcast(fp32))
                nc.sync.dma_start(
                    out=sqn, in_=dview(pk, OFF_PT + blk * 6144 + 4096,
                                       [[2048, 1], [1, 2048]]).bitcast(fp32))
                nc.scalar.activation(out=PM2, in_=PXYb, func=AF.Copy, scale=-2.0)
                for j in range(8):
                    g = blk * 8 + j
                    psD = PS.tile([128, N], fp32, tag="ps")
                    nc.tensor.matmul(psD, PM2[:, j, :], PM2[:, j, :],
                                     start=True, stop=False)
                    nc.tensor.matmul(psD, ONES[0:1, :], sqn[0:1, j, :],
                                     start=False, stop=True)
                    topk_jet(psD, g, 1.0)

            # cast idx and bounce through DRAM in jet-major layout
            def idx_to_dram(idxd):
                nc.vector.tensor_copy(out=Idx16, in_=IdxBig[:, :, 1:21])
                dst = dview(idxd, 0, [[K, N], [N * K, J], [1, K]])
                nc.sync.dma_start(out=dst, in_=Idx16[:, :, :])

            idx_to_dram(idxd1)
            for grp in range(G1):
                for k in range(NG1):
                    src = dview(idxd1, (NG1 * grp + k) * N * K,
                                [[0, 2], [8 * K, 16], [K, 8], [1, K]])
                    nc.sync.dma_start(
                        out=IdxW1[k * 32:(k + 1) * 32, grp, :], in_=src)

            # ---- conv1 L1 (batched over 4-jet strided chunks) ----
            FT = P.tile([16, J, N], fp16, tag="bigA")
            nc.sync.dma_start(out=FT, in_=dview(pk, 0, [[J * N + 64, 16], [1, J * N]]))
            for k in range(NG1):
                for c in range(4):
                    ftv = FT[:, 16 * c + k, :]
                    rhs = sview(ftv, 0, [[NG1 * N, 4], [1, N]])
                    psA = PS3.tile([128, 512], fp32, tag="psh")
                    psB = PS3.tile([128, 512], fp32, tag="psh")
                    nc.tensor.matmul(psA[k * 32:(k + 1) * 32, :], w1ah, rhs,
                                     start=True, stop=True, tile_position=(0, k * 32))
                    nc.tensor.matmul(psB[k * 32:(k + 1) * 32, :], w1bh, rhs,
                                     start=True, stop=True, tile_position=(0, k * 32))
                    nc.scalar.copy(out=Bm1_all[k * 32:(k + 1) * 32, 4 * c:4 * c + 4, :],
                                   in_=psB[k * 32:(k + 1) * 32, :])
                    nc.vector.tensor_sub(
                        out=SH1_all[k * 32:(k + 1) * 32, 4 * c:4 * c + 4, :],
                        in0=psA[k * 32:(k + 1) * 32, :],
                        in1=Bm1_all[k * 32:(k + 1) * 32, 4 * c:4 * c + 4, :])

            def edge_pass(src, SH, IdxW, ngrp, mode, s1=None, t1=None,
                          s2=None, t2=None, stats_t=None, wfold=None,
                          xout=None, b3ap=None):
                for grp in range(ngrp):
                    gat = WG.tile([128, E], fp32, tag="gat")
                    nc.gpsimd.ap_gather(
                        out_ap=gat[:, :], in_ap=src[:, grp, :],
                        idxs_ap=IdxW[:, grp, :],
                        channels=128, num_elems=N, d=1, num_idxs=E)
                    shv = SH[:, grp, :]
                    sh_b = sview(shv, 0, [[1, 8], [0, 20], [8, 16]])
                    g4 = gat.rearrange("p (a b q) -> p a b q", b=20, q=16)
                    nc.vector.tensor_add(out=g4, in0=g4, in1=sh_b)
                    if mode == "stats1":
                        for c in range(5):
                            nc.vector.bn_stats(
                                out=stats_t[:, grp * 5 + c, :],
                                in_=gat[:, c * 512:(c + 1) * 512])
                        continue
                    r1 = P.tile([128, E], fp32, tag="r1")
                    nc.scalar.activation(out=r1, in_=gat, func=AF.Relu,
                                         bias=t1, scale=s1)

                    def mean_fold(rr, psx):
                        # sum over neighbor dim b (strided view, b innermost)
                        rv = rr[:, :]
                        red = W.tile([128, N], fp32, tag="red")
                        rin = sview(rv, 0, [[320, 8], [1, 16], [16, 20]])
                        rout = sview(red[:, :], 0, [[16, 8], [1, 16]])
                        nc.vector.tensor_reduce(out=rout, in_=rin,
                                                axis=AX.X, op=OP.add)
                        nc.tensor.matmul(psx, wfold, red, start=True, stop=True)

                    if mode == "final2":
                        psx = PS2.tile([128, N], fp32, tag="psx")
                        mean_fold(r1, psx)
                        pm = S.tile([128, 1], fp32, tag="pm")
                        nc.vector.tensor_reduce(out=pm, in_=psx,
                                                axis=AX.X, op=OP.max)
                        nc.vector.tensor_scalar(out=pooled[:, grp:grp + 1], in0=pm,
                                                scalar1=1.0 / K, scalar2=b3ap,
                                                op0=OP.mult, op1=OP.add)
                        continue
                    r2 = None
                    if mode == "final1":
                        r2 = P.tile([128, E], fp32, tag="r2")
                    for c in range(5):
                        p = PS3.tile([128, 512], fp32, tag="psh")
                        nc.tensor.matmul(p, w2bd_s, r1[:, c * 512:(c + 1) * 512],
                                         start=True, stop=True)
                        if mode == "stats2":
                            nc.vector.bn_stats(out=stats_t[:, grp * 5 + c, :],
                                               in_=p)
                        else:
                            nc.scalar.activation(out=r2[:, c * 512:(c + 1) * 512],
                                                 in_=p, func=AF.Relu,
                                                 bias=t2, scale=s2)
                    if mode == "stats2":
                        continue
                    psx = PS2.tile([128, N], fp32, tag="psx")
                    mean_fold(r2, psx)
                    nc.vector.tensor_scalar(out=xout[:, grp, :], in0=psx,
                                            scalar1=1.0 / K, scalar2=b3ap,
                                            op0=OP.mult, op1=OP.add)

            def bn_param(stats_t, nchunk, foldm, gam, bet, cci, cco, nunits):
                mv = S.tile([128, 2], fp32, tag="mv")
                if nchunk <= 80:
                    nc.vector.bn_aggr(out=mv, in_=stats_t[:, 0:nchunk, :])
                else:
                    h = nchunk // 2
                    mv1 = S.tile([128, 2], fp32, tag="mv1")
                    mv2 = S.tile([128, 2], fp32, tag="mv2")
                    nc.vector.bn_aggr(out=mv1, in_=stats_t[:, 0:h, :])
                    nc.vector.bn_aggr(out=mv2, in_=stats_t[:, h:nchunk, :])
                    d = S.tile([128, 1], fp32, tag="mvd")
                    nc.vector.tensor_sub(out=d, in0=mv1[:, 0:1], in1=mv2[:, 0:1])
                    nc.vector.tensor_scalar_mul(out=d, in0=d, scalar1=0.5)
                    nc.vector.tensor_mul(out=d, in0=d, in1=d)
                    nc.vector.tensor_add(out=mv[:, 0:1], in0=mv1[:, 0:1], in1=mv2[:, 0:1])
                    nc.vector.tensor_scalar_mul(out=mv[:, 0:1], in0=mv[:, 0:1], scalar1=0.5)
                    nc.vector.tensor_add(out=mv[:, 1:2], in0=mv1[:, 1:2], in1=mv2[:, 1:2])
                    nc.vector.tensor_scalar(out=mv[:, 1:2], in0=mv[:, 1:2],
                                            scalar1=0.5, scalar2=None, op0=OP.mult)
                    nc.vector.tensor_add(out=mv[:, 1:2], in0=mv[:, 1:2], in1=d)
                pay = S.tile([128, 4], fp32, tag="pay")
                nc.vector.tensor_copy(out=pay[:, 0:2], in_=mv)
                nc.vector.tensor_mul(out=pay[:, 2:3], in0=mv[:, 0:1], in1=mv[:, 0:1])
                nc.vector.memset(pay[:, 3:4], 0.0)
                if USE_ALLREDUCE:
                    nc.gpsimd.dma_start(out=cci[:, :], in_=pay)
                    nc.gpsimd.collective_compute(
                        "AllReduce", OP.add,
                        replica_groups=[list(range(N_CORES))],
                        ins=[cci[:, :]], outs=[cco[:, :]])
                    arr = S.tile([128, 4], fp32, tag="arr")
                    nc.gpsimd.dma_start(out=arr, in_=cco[:, :])
                else:
                    arr = pay
                psf = PS.tile([128, 4], fp32, tag="ps")
                nc.tensor.matmul(psf, foldm, arr, start=True, stop=True)
                mg = S.tile([128, 1], fp32, tag="mg")
                vg = S.tile([128, 1], fp32, tag="vg")
                nc.vector.tensor_scalar_mul(out=mg, in0=psf[:, 0:1], scalar1=1.0 / nunits)
                m2g = S.tile([128, 1], fp32, tag="m2g")
                nc.vector.tensor_scalar_mul(out=m2g, in0=psf[:, 2:3], scalar1=1.0 / nunits)
                nc.vector.tensor_scalar_mul(out=vg, in0=psf[:, 1:2], scalar1=1.0 / nunits)
                nc.vector.tensor_add(out=vg, in0=vg, in1=m2g)
                mm = S.tile([128, 1], fp32, tag="mm")
                nc.vector.tensor_mul(out=mm, in0=mg, in1=mg)
                nc.vector.tensor_sub(out=vg, in0=vg, in1=mm)
                sd = S.tile([128, 1], fp32, tag="sd")
                nc.scalar.activation(out=sd, in_=vg, func=AF.Sqrt, bias=epsap, scale=1.0)
                ri = S.tile([128, 1], fp32, tag="ri")
                nc.vector.reciprocal(out=ri, in_=sd)
                s = P.tile([128, 1], fp32, tag=f"bn_s_{cci.name}")
                t = P.tile([128, 1], fp32, tag=f"bn_t_{cci.name}")
                nc.vector.tensor_mul(out=s, in0=gam, in1=ri)
                nc.vector.tensor_mul(out=t, in0=mg, in1=s)
                nc.vector.tensor_sub(out=t, in0=bet, in1=t)
                return s, t

            # ---- conv1 stats + passes ----
            statsA = ST.tile([128, G2 * 5, 6], fp32, tag="stats")
            stats1 = statsA[:, 0:G1 * 5, :]
            edge_pass(Bm1_all, SH1_all, IdxW1, G1, "stats1", stats_t=stats1)
            s1, t1 = bn_param(stats1, G1 * 5, fold4_s, g1r_s, be1r_s,
                              cc_in[0], cc_out[0],
                              4 * N_CORES if USE_ALLREDUCE else 4)
            statsB = ST.tile([128, G2 * 5, 6], fp32, tag="stats")
            stats2 = statsB[:, 0:G1 * 5, :]
            edge_pass(Bm1_all, SH1_all, IdxW1, G1, "stats2", s1=s1, t1=t1,
                      stats_t=stats2)
            s2, t2 = bn_param(stats2, G1 * 5, fold4_s, g2r_s, be2r_s,
                              cc_in[1], cc_out[1],
                              4 * N_CORES if USE_ALLREDUCE else 4)
            edge_pass(Bm1_all, SH1_all, IdxW1, G1, "final1", s1=s1, t1=t1,
                      s2=s2, t2=t2, wfold=w3bd_s, xout=x1_all, b3ap=b3r_s)

            # ---- conv2 prep: -2x and banded squared norms ----
            X2 = P.tile([128, G1, N], fp32, tag="bigA")
            sqx = P.tile([128, G1, N], fp32, tag="sqx")
            sqn_s = P.tile([128, G1, N], fp32, tag="sqn_s")
            nc.scalar.activation(out=X2, in_=x1_all, func=AF.Copy, scale=-2.0)
            nc.vector.tensor_mul(out=sqx, in0=x1_all, in1=x1_all)
            for c in range(4):
                pss = PS3.tile([128, 512], fp32, tag="psh")
                nc.tensor.matmul(pss, blk4s, sqx[:, 4 * c:4 * c + 4, :],
                                 start=True, stop=True)
                nc.scalar.copy(out=sqn_s[:, 4 * c:4 * c + 4, :], in_=pss)

            # ---- conv2 kNN ----
            for g in range(J):
                k = g % NG1
                grp = g // NG1
                psD = PS.tile([128, N], fp32, tag="ps")
                nc.tensor.matmul(psD, X2[k * 32:(k + 1) * 32, grp, :],
                                 x1_all[k * 32:(k + 1) * 32, grp, :],
                                 start=True, stop=False,
                                 tile_position=(k * 32, 0))
                nc.tensor.matmul(psD, ONES[k * 32:k * 32 + 1, :],
                                 sqn_s[k * 32:k * 32 + 1, grp, :],
                                 start=False, stop=True,
                                 tile_position=(k * 32, 0))
                topk_jet(psD, g, -1.0)

            idx_to_dram(idxd2)
            for grp2 in range(G2):
                for k2 in range(NG2):
                    src = dview(idxd2, (NG2 * grp2 + k2) * N * K,
                                [[0, 4], [8 * K, 16], [K, 8], [1, K]])
                    nc.sync.dma_start(
                        out=IdxW2[k2 * 64:(k2 + 1) * 64, grp2, :], in_=src)

            # ---- conv2 L1 (batched) ----
            for k1 in range(4):
                b2 = (k1 % 2) * 64
                go = k1 // 2
                for c in range(4):
                    psA = PS3.tile([128, 512], fp32, tag="psh")
                    psB = PS3.tile([128, 512], fp32, tag="psh")
                    rhs = x1_all[k1 * 32:(k1 + 1) * 32, 4 * c:4 * c + 4, :]
                    nc.tensor.matmul(psA[b2:b2 + 64, :],
                                     W2PA4[k1 * 32:(k1 + 1) * 32, :], rhs,
                                     start=True, stop=True,
                                     tile_position=(k1 * 32, b2))
                    nc.tensor.matmul(psB[b2:b2 + 64, :],
                                     W2PB4[k1 * 32:(k1 + 1) * 32, :], rhs,
                                     start=True, stop=True,
                                     tile_position=(k1 * 32, b2))
                    b2v = B2_all[b2:b2 + 64, 0, :]
                    dstB = sview(b2v, (8 * c + go) * N, [[2 * N, 4], [1, N]])
                    s2v = SH2_all[b2:b2 + 64, 0, :]
                    dstS = sview(s2v, (8 * c + go) * N, [[2 * N, 4], [1, N]])
                    nc.scalar.copy(out=dstB, in_=psB[b2:b2 + 64, :])
                    nc.vector.tensor_sub(out=dstS, in0=psA[b2:b2 + 64, :], in1=dstB)

            # ---- conv2 stats + final ----
            stats3 = ST.tile([128, G2 * 5, 6], fp32, tag="stats")
            edge_pass(B2_all, SH2_all, IdxW2, G2, "stats1", stats_t=stats3)
            s3, t3 = bn_param(stats3, G2 * 5, fold2_s, g3r_s, be3r_s,
                              cc_in[2], cc_out[2],
                              2 * N_CORES if USE_ALLREDUCE else 2)
            edge_pass(B2_all, SH2_all, IdxW2, G2, "final2", s1=s3, t1=t3,
                      wfold=w2pbd_s, b3ap=b2pr_s)

            # ---- head (fp16 weights/activations) ----
            pooledh = W.tile([128, G2], fp16, tag="pooledh")
            nc.vector.tensor_copy(out=pooledh, in_=pooled)
            Gh = P.tile([64, J], fp16, tag="Gh")
            gh_v = Gh.rearrange("p (g s) -> p g s", s=2)
            nc.sync.dma_start(out=gh_v[:, :, 0], in_=pooledh[0:64, :])
            nc.sync.dma_start(out=gh_v[:, :, 1], in_=pooledh[64:128, :])
            ps1 = PS.tile([128, J], fp32, tag="ps")
            nc.tensor.matmul(ps1, mh1_s, Gh, start=True, stop=True)
            hh1 = W.tile([128, J], fp16, tag="hh1")
            nc.scalar.activation(out=hh1, in_=ps1, func=AF.Relu, bias=mb1_s, scale=1.0)
            ps2 = PS.tile([128, J], fp32, tag="ps")
            nc.tensor.matmul(ps2, mh2_s, hh1, start=True, stop=True)
            hh2 = W.tile([128, J], fp16, tag="hh2")
            nc.scalar.activation(out=hh2, in_=ps2, func=AF.Relu, bias=mb2_s, scale=1.0)
            ps3 = PS.tile([8, J], fp32, tag="ps")
            nc.tensor.matmul(ps3, mh3_s, hh2, start=True, stop=True)
            ov = W.tile([1, J], fp32, tag="ov")
            nc.vector.tensor_scalar(out=ov, in0=ps3[0:1, :], scalar1=mb3_s[0:1, 0:1],
                                    scalar2=None, op0=OP.add)
            nc.sync.dma_start(out=out_t[:, :], in_=ov)

    nc.finalize()
    return nc


_NC_CACHE = None
_CACHE_SET = False
LAST_EXEC_NS = None


def _enable_jax_cache():
    global _CACHE_SET
    if _CACHE_SET:
        return
    import jax
    jax.config.update("jax_compilation_cache_dir", "/tmp/bass_jax_cache_v2")
    jax.config.update("jax_persistent_cache_min_compile_time_secs", 0.0)
    jax.config.update("jax_persistent_cache_min_entry_size_bytes", 0)
    _CACHE_SET = True


def _pack_weights(i):
    wp = np.zeros((128, WC), np.float32)
    wp[0:64, 0:64] = i["c2_w2"]
    wp[64:96, 0:64] = i["c2_w1"][:32]
    wp[96:128, 0:64] = i["c2_w1"][32:]
    wp[0:32, 64:96] = i["c1_w2"]
    wp[32:64, 64:96] = i["c1_w3"]
    wp[64:96, 64:96] = np.eye(32, dtype=np.float32)
    wp[0:128, 96] = np.tile(i["c1_g1"], 4)
    wp[0:128, 97] = np.tile(i["c1_be1"], 4)
    wp[0:128, 98] = np.tile(i["c1_g2"], 4)
    wp[0:128, 99] = np.tile(i["c1_be2"], 4)
    wp[0:128, 100] = np.tile(i["c1_b3"], 4)
    wp[0:128, 101] = np.tile(i["c2_g1"], 2)
    wp[0:128, 102] = np.tile(i["c2_be1"], 2)
    wp[0:128, 103] = np.tile(i["c2_b2"], 2)
    wp[0:128, 104] = i["m_b1"]
    wp[0:128, 105] = i["m_b2"]
    wp[0, 106] = i["m_b3"][0]
    mh = np.zeros((128, MC), np.float16)
    mh[0:128, 0:128] = i["m_w2"].astype(np.float16)
    mh[0:128, 128:192] = i["m_w1"].astype(np.float16).T
    mh[0:128, 192:193] = i["m_w3"].astype(np.float16)
    return wp, mh


def kernel(**inputs) -> np.ndarray:
    global _NC_CACHE, LAST_EXEC_NS
    _enable_jax_cache()
    from concourse.bass_utils import run_bass_kernel_spmd

    if _NC_CACHE is None:
        _NC_CACHE = _build_nc()
        # the module is immutable after finalize(); memoize its JSON so the
        # per-call jit lowering doesn't re-serialize 3MB of BIR every time
        _json = _NC_CACHE.to_json_bytes()
        _NC_CACHE.to_json_bytes = lambda _j=_json: _j
    nc = _NC_CACHE

    pts = inputs["points"].astype(np.float32)
    feat = inputs["features"].astype(np.float32)
    wp, mh = _pack_weights({k: np.asarray(v, np.float32) for k, v in inputs.items()
                            if k not in ("points", "features")})

    w1 = np.asarray(inputs["c1_w1"], np.float32)
    wh = np.concatenate([w1[:16], w1[16:]], axis=1).astype(np.float16)
    mh_flat = mh.reshape(-1)
    wp_bits = wp.reshape(-1).view(np.float16)
    # vectorized packing across all cores at once
    ft16 = feat.transpose(2, 0, 1).astype(np.float16)      # [16, B, N]
    d_all = np.empty((N_CORES, 3, J, N), np.float32)
    d_all[:, 0] = pts[:, :, 0].reshape(N_CORES, J, N)
    d_all[:, 1] = pts[:, :, 1].reshape(N_CORES, J, N)
    d_all[:, 2] = -2.0 * (pts[:, :, 0] ** 2 + pts[:, :, 1] ** 2).reshape(N_CORES, J, N)
    # per-block layout: (core, blk, row{x,y,sqn}, jet, node)
    dblk_all = np.ascontiguousarray(
        d_all.reshape(N_CORES, 3, 8, 8, N).transpose(0, 2, 1, 3, 4))
    dbits = dblk_all.reshape(N_CORES, -1).view(np.float16)
    in_maps = []
    for c in range(N_CORES):
        pkv = np.empty(PK_LEN, np.float16)
        fc2 = pkv[0:OFF_MH].reshape(16, J * N + 64)
        fc2[:, 0:J * N] = ft16[:, c * J:(c + 1) * J, :].reshape(16, J * N)
        fc2[:, J * N:] = wh
        pkv[OFF_MH:OFF_WP] = mh_flat
        pkv[OFF_WP:OFF_PT] = wp_bits
        pkv[OFF_PT:] = dbits[c]
        in_maps.append({"pk": pkv.reshape(1, PK_LEN)})

    import time as _t
    _t0 = _t.time()
    try:
        res = run_bass_kernel_spmd(nc, in_maps, core_ids=list(range(N_CORES)))
    except Exception:
        # transient device hiccup (e.g. NRT_EXEC_UNIT_UNRECOVERABLE): retry once
        _t0 = _t.time()
        res = run_bass_kernel_spmd(nc, in_maps, core_ids=list(range(N_CORES)))
    _t1 = _t.time()
    LAST_EXEC_NS = int((_t1 - _t0) * 1e9)
    import os
    if os.environ.get("KERNEL_TRACE", "0") == "1":
        print(f"HW exec time: {LAST_EXEC_NS} ns (wall of spmd execute)")
    outs = [res.results[c]["out"].reshape(J) for c in range(N_CORES)]
    return np.concatenate(outs).reshape(B, 1).astype(np.float32)
